# revision 11
# baseline (speedup 1.0000x reference)
"""Trainium2 Bass kernel for nn_Discriminator2 (bilinear discriminator scores).

Math: with hc0 = h_c[0] [N, D], W0 = W[0] [D, D]:
    v      = hc0 @ W0.T                      [N, D]   (tensor engine, bf16)
    sc1[n] = dot(h_pl[0][n], v[n]) + b       [N]
    sc2[s,n] = dot(hc0[sample[s,n]], v[n]) + b        (indirect-DMA gather)
    out    = [sc1 | sc2.flat | sc2.flat]     [1, N + 2*S*N]

Sharding: nodes (N) split evenly across 8 cores; hc0 replicated on every core
(bf16 copy) so gathers resolve locally; W replicated; h_pl / sample_list
sharded by node.

Data path is bf16 (tensors are host-cast); accumulations (PSUM, reduces,
output) are fp32. Structure per pair of 128-node tiles: 8 indirect gathers,
8 matmuls, one ACT cast-copy of v, two batched DVE multiplies and one batched
DVE reduce.
"""

import sys

for _p in ("/opt/trn_rl_repo",):
    if _p not in sys.path:
        sys.path.insert(0, _p)

import ml_dtypes
import numpy as np

import concourse.bass as bass
import concourse.mybir as mybir
import concourse.tile as tile
from concourse import bacc
from concourse.bass_utils import run_bass_kernel_spmd

P = 128  # partitions
BF16 = ml_dtypes.bfloat16


class Cfg:
    """Problem geometry. Full-size defaults; shrink for CoreSim validation."""

    def __init__(self, n_table=100000, nodes_per_core=12500, d=512, s=4,
                 n_cores=8, super_tile=4):
        self.n_table = n_table          # rows of the gather table (full N)
        self.nodes_per_core = nodes_per_core
        self.d = d
        self.s = s
        self.n_cores = n_cores
        self.super_tile = super_tile    # node-tiles per hcT DMA block (even)
        self.tiles = -(-nodes_per_core // (2 * P)) * 2   # ceil to even
        self.npad = self.tiles * P
        self.kc = d // P                # contraction chunks


FULL = Cfg()


def build_nc(cfg: Cfg):
    D, S, KC, TILES = cfg.d, cfg.s, cfg.kc, cfg.tiles
    f32 = mybir.dt.float32
    bf = mybir.dt.bfloat16
    NS = S + 1                          # dot slots per node: S samples + sc1

    nc = bacc.Bacc("TRN2", target_bir_lowering=False, debug=False)
    hc = nc.dram_tensor("hc", [cfg.n_table, D], bf, kind="ExternalInput").ap()
    hcT = nc.dram_tensor("hcT", [D, cfg.npad], bf, kind="ExternalInput").ap()
    hpl = nc.dram_tensor("hpl", [cfg.npad, D], bf, kind="ExternalInput").ap()
    idx = nc.dram_tensor("idx", [P, TILES * S], mybir.dt.int32,
                         kind="ExternalInput").ap()
    wt = nc.dram_tensor("wt", [D, D], bf, kind="ExternalInput").ap()
    bb = nc.dram_tensor("bb", [P, 1], f32, kind="ExternalInput").ap()
    out = nc.dram_tensor("out", [P, TILES * NS], f32,
                         kind="ExternalOutput").ap()

    with tile.TileContext(nc) as tc:
        with (
            tc.tile_pool(name="const", bufs=1) as cpool,
            tc.tile_pool(name="hcT", bufs=2) as hcT_pool,
            tc.tile_pool(name="hpl", bufs=3) as hpl_pool,
            tc.tile_pool(name="g", bufs=3) as g_pool,
            tc.tile_pool(name="v", bufs=3) as v_pool,
            tc.tile_pool(name="prod", bufs=2) as prod_pool,
            tc.tile_pool(name="psum", bufs=3, space="PSUM") as psum_pool,
        ):
            # W.T resident: free layout (c, d) — chunk c covers contraction
            # rows c*128..c*128+127.
            wt_sb = cpool.tile([P, KC * D], bf)
            nc.sync.dma_start(
                out=wt_sb[:].rearrange("p (c d) -> p c d", c=KC),
                in_=wt.rearrange("(c p) d -> p c d", p=P))
            # All gather indices resident: idx_sb[p, t*S+s] = sample[s, t*128+p].
            idx_sb = cpool.tile([P, TILES * S], mybir.dt.int32)
            nc.sync.dma_start(out=idx_sb[:], in_=idx[:])
            b_sb = cpool.tile([P, 1], f32)
            nc.sync.dma_start(out=b_sb[:], in_=bb[:])
            sc_acc = cpool.tile([P, TILES * NS], f32)

            for t0 in range(0, TILES, cfg.super_tile):
                st = min(cfg.super_tile, TILES - t0)
                # hcT block [D, st*128] -> SBUF free layout (c, n_local)
                hcT_sb = hcT_pool.tile([P, KC * cfg.super_tile * P], bf,
                                       tag="hcT")
                nc.sync.dma_start(
                    out=hcT_sb[:, : KC * st * P].rearrange(
                        "p (c n) -> p c n", c=KC),
                    in_=hcT[:, t0 * P:(t0 + st) * P].rearrange(
                        "(c p) n -> p c n", p=P),
                )
                hcT_3d = hcT_sb[:, : KC * st * P].rearrange(
                    "p (c n) -> p c n", c=KC)
                for jj in range(st // 2):          # pairs within the block
                    tp = t0 + 2 * jj               # first tile of the pair
                    # h_pl for 256 nodes: [128, (j, d)]
                    hpl_sb = hpl_pool.tile([P, 2 * D], bf, tag="hpl")
                    nc.sync.dma_start(
                        out=hpl_sb[:].rearrange("p (j d) -> p j d", j=2),
                        in_=hpl[tp * P:(tp + 2) * P, :].rearrange(
                            "(j p) d -> p j d", p=P))
                    # 8 gathers: g_sb[p, ((j, s), d)] = hc[idx[s, (tp+j)*128+p]]
                    g_sb = g_pool.tile([P, 2 * S * D], bf, tag="g")
                    for j in range(2):
                        for s in range(S):
                            col = (tp + j) * S + s
                            nc.gpsimd.indirect_dma_start(
                                out=g_sb[:, (j * S + s) * D:
                                         (j * S + s + 1) * D],
                                out_offset=None,
                                in_=hc[:],
                                in_offset=bass.IndirectOffsetOnAxis(
                                    ap=idx_sb[:, col:col + 1], axis=0),
                            )
                    # v for both tiles: [128, (j, d)] fp32 in PSUM (2 banks)
                    v_ps = psum_pool.tile([P, 2 * D], f32, space="PSUM",
                                          tag="v_ps")
                    for j in range(2):
                        for c in range(KC):
                            nc.tensor.matmul(
                                out=v_ps[:, j * D:(j + 1) * D],
                                lhsT=hcT_3d[:, c,
                                            (2 * jj + j) * P:
                                            (2 * jj + j + 1) * P],
                                rhs=wt_sb[:, c * D:(c + 1) * D],
                                start=(c == 0),
                                stop=(c == KC - 1),
                            )
                    # ACT cast-copy: v fp32 PSUM -> bf16 SBUF (one op per
                    # PSUM bank — a single op must not cross banks)
                    v_sb = v_pool.tile([P, 2 * D], bf, tag="v")
                    for j in range(2):
                        nc.scalar.copy(v_sb[:, j * D:(j + 1) * D],
                                       v_ps[:, j * D:(j + 1) * D])
                    # products, layout [128, (j, slot, d)], slots 0..3 = s,
                    # slot 4 = sc1
                    prod = prod_pool.tile([P, 2 * NS * D], bf, tag="prod")
                    prod_4d = prod[:].rearrange("p (j s d) -> p j s d",
                                                j=2, s=NS)
                    v_bcast = (v_sb[:].rearrange("p (j x d) -> p j x d",
                                                 j=2, x=1)
                               .to_broadcast([P, 2, S, D]))
                    nc.vector.tensor_mul(
                        prod_4d[:, :, 0:S, :],
                        g_sb[:].rearrange("p (j s d) -> p j s d", j=2, s=S),
                        v_bcast)
                    nc.vector.tensor_mul(
                        prod_4d[:, :, S, :],
                        hpl_sb[:].rearrange("p (j d) -> p j d", j=2),
                        v_sb[:].rearrange("p (j d) -> p j d", j=2))
                    # one batched reduce -> 10 dot results, straight into
                    # sc_acc slots [tp*NS, tp*NS + 2*NS)
                    nc.vector.reduce_sum(
                        sc_acc[:, tp * NS:(tp + 2) * NS],
                        prod[:].rearrange("p (k d) -> p k d", d=D),
                        axis=mybir.AxisListType.X)
            nc.vector.tensor_scalar_add(sc_acc[:], sc_acc[:], b_sb[:, :1])
            nc.sync.dma_start(out=out[:], in_=sc_acc[:])
    nc.compile()
    return nc


def make_in_maps(cfg: Cfg, h_c, h_pl, sample_list, W, b):
    """Host-side sharding: full inputs -> per-core input dicts."""
    D, S = cfg.d, cfg.s
    hc0 = np.asarray(h_c, np.float32)[0]
    hpl0 = np.asarray(h_pl, np.float32)[0]
    smp = np.asarray(sample_list)
    W0 = np.asarray(W, np.float32)[0]
    bval = float(np.asarray(b, np.float32).reshape(-1)[0])

    hc_bf = np.ascontiguousarray(hc0.astype(BF16))
    hcT_bf = np.ascontiguousarray(hc0.T.astype(BF16))       # [D, N]
    wt_bf = np.ascontiguousarray(W0.T.astype(BF16))         # wt[e, d] = W[d, e]
    b_bcast = np.full((P, 1), bval, np.float32)

    in_maps = []
    for c in range(cfg.n_cores):
        lo = c * cfg.nodes_per_core
        hi = lo + cfg.nodes_per_core
        hcT_s = np.zeros((D, cfg.npad), BF16)
        hcT_s[:, : cfg.nodes_per_core] = hcT_bf[:, lo:hi]
        hpl_s = np.zeros((cfg.npad, D), BF16)
        hpl_s[: cfg.nodes_per_core] = hpl0[lo:hi].astype(BF16)
        idx_s = np.zeros((S, cfg.npad), np.int64)
        idx_s[:, : cfg.nodes_per_core] = smp[:, lo:hi]
        idx_r = np.ascontiguousarray(
            idx_s.reshape(S, cfg.tiles, P).transpose(2, 1, 0)
            .astype(np.int32).reshape(P, cfg.tiles * S))
        in_maps.append({
            "hc": hc_bf, "hcT": hcT_s, "hpl": hpl_s,
            "idx": idx_r, "wt": wt_bf, "bb": b_bcast,
        })
    return in_maps


def assemble_output(cfg: Cfg, outs):
    """Per-core 'out' arrays [P, TILES*(S+1)] -> full logits [1, N + 2*S*N].

    Slot order within a tile: 0..S-1 = samples, S = sc1.
    """
    S = cfg.s
    n = cfg.nodes_per_core * cfg.n_cores
    sc1 = np.empty((n,), np.float32)
    sc2 = np.empty((S, n), np.float32)
    for c in range(cfg.n_cores):
        o = (outs[c].reshape(P, cfg.tiles, S + 1).transpose(2, 1, 0)
             .reshape(S + 1, cfg.npad)[:, : cfg.nodes_per_core])
        lo = c * cfg.nodes_per_core
        sc1[lo:lo + cfg.nodes_per_core] = o[S]
        sc2[:, lo:lo + cfg.nodes_per_core] = o[:S]
    flat = sc2.reshape(-1)
    return np.concatenate([sc1, flat, flat])[None, :].astype(np.float32)


_NC_CACHE = {}


def _get_nc(cfg: Cfg):
    key = (cfg.n_table, cfg.nodes_per_core, cfg.d, cfg.s, cfg.super_tile)
    if key not in _NC_CACHE:
        _NC_CACHE[key] = build_nc(cfg)
    return _NC_CACHE[key]


def run_on_hw(cfg: Cfg, inputs, trace=False, trace_kwargs={}):
    nc = _get_nc(cfg)
    in_maps = make_in_maps(cfg, **inputs)
    res = run_bass_kernel_spmd(nc, in_maps, core_ids=list(range(cfg.n_cores)),
                               trace=trace, trace_kwargs=trace_kwargs)
    out = assemble_output(cfg, [r["out"] for r in res.results])
    return out, res


def kernel(h_c, h_pl, sample_list, W, b):
    inputs = dict(h_c=h_c, h_pl=h_pl, sample_list=sample_list, W=W, b=b)
    out, _ = run_on_hw(FULL, inputs, trace=False)
    return out


# revision 16
# speedup vs baseline: 1.0277x; 1.0277x over previous
"""Trainium2 Bass kernel for nn_Discriminator2 (bilinear discriminator scores).

Math: with hc0 = h_c[0] [N, D], W0 = W[0] [D, D]:
    v      = hc0 @ W0.T                      [N, D]   (tensor engine, bf16)
    sc1[n] = dot(h_pl[0][n], v[n]) + b       [N]
    sc2[s,n] = dot(hc0[sample[s,n]], v[n]) + b        (indirect-DMA gather)
    out    = [sc1 | sc2.flat | sc2.flat]     [1, N + 2*S*N]

Sharding: nodes (N) split evenly across 8 cores; hc0 replicated on every core
(bf16 copy) so gathers resolve locally; W replicated; h_pl / sample_list
sharded by node.

Data path is bf16 (tensors are host-cast); accumulations (PSUM, reduces,
output) are fp32. Structure per pair of 128-node tiles: 8 indirect gathers,
8 matmuls, one ACT cast-copy of v, two batched DVE multiplies and one batched
DVE reduce.
"""

import sys

for _p in ("/opt/trn_rl_repo",):
    if _p not in sys.path:
        sys.path.insert(0, _p)

import ml_dtypes
import numpy as np

import concourse.bass as bass
import concourse.mybir as mybir
import concourse.tile as tile
from concourse import bacc
from concourse.bass_utils import run_bass_kernel_spmd

P = 128  # partitions
BF16 = ml_dtypes.bfloat16


class Cfg:
    """Problem geometry. Full-size defaults; shrink for CoreSim validation."""

    def __init__(self, n_table=100000, nodes_per_core=12500, d=512, s=4,
                 n_cores=8, super_tile=4):
        self.n_table = n_table          # rows of the gather table (full N)
        self.nodes_per_core = nodes_per_core
        self.d = d
        self.s = s
        self.n_cores = n_cores
        self.super_tile = super_tile    # node-tiles per hcT DMA block (even)
        self.tiles = -(-nodes_per_core // (2 * P)) * 2   # ceil to even
        self.npad = self.tiles * P
        self.kc = d // P                # contraction chunks


FULL = Cfg()


def build_nc(cfg: Cfg):
    D, S, KC, TILES = cfg.d, cfg.s, cfg.kc, cfg.tiles
    f32 = mybir.dt.float32
    bf = mybir.dt.bfloat16
    NS = S + 1                          # dot slots per node: S samples + sc1

    nc = bacc.Bacc("TRN2", target_bir_lowering=False, debug=False)
    hc = nc.dram_tensor("hc", [cfg.n_table, D], bf, kind="ExternalInput").ap()
    hcT = nc.dram_tensor("hcT", [D, cfg.npad], bf, kind="ExternalInput").ap()
    hpl = nc.dram_tensor("hpl", [cfg.npad, D], bf, kind="ExternalInput").ap()
    idx = nc.dram_tensor("idx", [P, TILES * S], mybir.dt.int32,
                         kind="ExternalInput").ap()
    wt = nc.dram_tensor("wt", [D, D], bf, kind="ExternalInput").ap()
    bb = nc.dram_tensor("bb", [P, 1], f32, kind="ExternalInput").ap()
    out = nc.dram_tensor("out", [P, TILES * NS], f32,
                         kind="ExternalOutput").ap()

    with tile.TileContext(nc) as tc:
        with (
            tc.tile_pool(name="const", bufs=1) as cpool,
            tc.tile_pool(name="hcT", bufs=2) as hcT_pool,
            tc.tile_pool(name="g", bufs=4) as g_pool,
            tc.tile_pool(name="v", bufs=4) as v_pool,
            tc.tile_pool(name="prod", bufs=3) as prod_pool,
            tc.tile_pool(name="psum", bufs=3, space="PSUM") as psum_pool,
        ):
            # W.T resident: free layout (c, d) — chunk c covers contraction
            # rows c*128..c*128+127.
            wt_sb = cpool.tile([P, KC * D], bf)
            nc.sync.dma_start(
                out=wt_sb[:].rearrange("p (c d) -> p c d", c=KC),
                in_=wt.rearrange("(c p) d -> p c d", p=P))
            # All gather indices resident: idx_sb[p, t*S+s] = sample[s, t*128+p].
            idx_sb = cpool.tile([P, TILES * S], mybir.dt.int32)
            nc.sync.dma_start(out=idx_sb[:], in_=idx[:])
            b_sb = cpool.tile([P, 1], f32)
            nc.sync.dma_start(out=b_sb[:], in_=bb[:])
            sc_acc = cpool.tile([P, TILES * NS], f32)

            for t0 in range(0, TILES, cfg.super_tile):
                st = min(cfg.super_tile, TILES - t0)
                # hcT block [D, st*128] -> SBUF free layout (c, n_local)
                hcT_sb = hcT_pool.tile([P, KC * cfg.super_tile * P], bf,
                                       tag="hcT")
                nc.sync.dma_start(
                    out=hcT_sb[:, : KC * st * P].rearrange(
                        "p (c n) -> p c n", c=KC),
                    in_=hcT[:, t0 * P:(t0 + st) * P].rearrange(
                        "(c p) n -> p c n", p=P),
                )
                hcT_3d = hcT_sb[:, : KC * st * P].rearrange(
                    "p (c n) -> p c n", c=KC)
                for jj in range(st // 2):          # pairs within the block
                    tp = t0 + 2 * jj               # first tile of the pair
                    # g_sb free layout (j, slot, d): slots 0..S-1 = gathered
                    # sample rows, slot S = this tile's h_pl rows. One wide
                    # buffer so a single DVE multiply covers all 5 dots.
                    g_sb = g_pool.tile([P, 2 * NS * D], bf, tag="g")
                    nc.sync.dma_start(
                        out=g_sb[:].rearrange("p (j s d) -> p j s d",
                                              j=2, s=NS)[:, :, S, :],
                        in_=hpl[tp * P:(tp + 2) * P, :].rearrange(
                            "(j p) d -> p j d", p=P))
                    for j in range(2):
                        for s in range(S):
                            col = (tp + j) * S + s
                            nc.gpsimd.indirect_dma_start(
                                out=g_sb[:, (j * NS + s) * D:
                                         (j * NS + s + 1) * D],
                                out_offset=None,
                                in_=hc[:],
                                in_offset=bass.IndirectOffsetOnAxis(
                                    ap=idx_sb[:, col:col + 1], axis=0),
                            )
                    # v for both tiles: [128, (j, d)] fp32 in PSUM (2 banks)
                    v_ps = psum_pool.tile([P, 2 * D], f32, space="PSUM",
                                          tag="v_ps")
                    for j in range(2):
                        for c in range(KC):
                            nc.tensor.matmul(
                                out=v_ps[:, j * D:(j + 1) * D],
                                lhsT=hcT_3d[:, c,
                                            (2 * jj + j) * P:
                                            (2 * jj + j + 1) * P],
                                rhs=wt_sb[:, c * D:(c + 1) * D],
                                start=(c == 0),
                                stop=(c == KC - 1),
                            )
                    # ACT cast-copy: v fp32 PSUM -> bf16 SBUF (one op per
                    # PSUM bank — a single op must not cross banks)
                    v_sb = v_pool.tile([P, 2 * D], bf, tag="v")
                    for j in range(2):
                        nc.scalar.copy(v_sb[:, j * D:(j + 1) * D],
                                       v_ps[:, j * D:(j + 1) * D])
                    # one multiply for all 10 dot slots (v broadcast over
                    # the 5 slots of each tile)
                    prod = prod_pool.tile([P, 2 * NS * D], bf, tag="prod")
                    v_bcast = (v_sb[:].rearrange("p (j x d) -> p j x d",
                                                 j=2, x=1)
                               .to_broadcast([P, 2, NS, D]))
                    nc.vector.tensor_mul(
                        prod[:].rearrange("p (j s d) -> p j s d", j=2, s=NS),
                        g_sb[:].rearrange("p (j s d) -> p j s d", j=2, s=NS),
                        v_bcast)
                    # 2D reduces (4x DVE mode; the batched 3D form falls to
                    # 1x) -> 10 dot results into sc_acc slots
                    for k in range(2 * NS):
                        nc.vector.reduce_sum(
                            sc_acc[:, tp * NS + k:tp * NS + k + 1],
                            prod[:, k * D:(k + 1) * D],
                            axis=mybir.AxisListType.X)
            nc.vector.tensor_scalar_add(sc_acc[:], sc_acc[:], b_sb[:, :1])
            nc.sync.dma_start(out=out[:], in_=sc_acc[:])
    nc.compile()
    return nc


def make_in_maps(cfg: Cfg, h_c, h_pl, sample_list, W, b):
    """Host-side sharding: full inputs -> per-core input dicts."""
    D, S = cfg.d, cfg.s
    hc0 = np.asarray(h_c, np.float32)[0]
    hpl0 = np.asarray(h_pl, np.float32)[0]
    smp = np.asarray(sample_list)
    W0 = np.asarray(W, np.float32)[0]
    bval = float(np.asarray(b, np.float32).reshape(-1)[0])

    hc_bf = np.ascontiguousarray(hc0.astype(BF16))
    hcT_bf = np.ascontiguousarray(hc0.T.astype(BF16))       # [D, N]
    wt_bf = np.ascontiguousarray(W0.T.astype(BF16))         # wt[e, d] = W[d, e]
    b_bcast = np.full((P, 1), bval, np.float32)

    in_maps = []
    for c in range(cfg.n_cores):
        lo = c * cfg.nodes_per_core
        hi = lo + cfg.nodes_per_core
        hcT_s = np.zeros((D, cfg.npad), BF16)
        hcT_s[:, : cfg.nodes_per_core] = hcT_bf[:, lo:hi]
        hpl_s = np.zeros((cfg.npad, D), BF16)
        hpl_s[: cfg.nodes_per_core] = hpl0[lo:hi].astype(BF16)
        idx_s = np.zeros((S, cfg.npad), np.int64)
        idx_s[:, : cfg.nodes_per_core] = smp[:, lo:hi]
        idx_r = np.ascontiguousarray(
            idx_s.reshape(S, cfg.tiles, P).transpose(2, 1, 0)
            .astype(np.int32).reshape(P, cfg.tiles * S))
        in_maps.append({
            "hc": hc_bf, "hcT": hcT_s, "hpl": hpl_s,
            "idx": idx_r, "wt": wt_bf, "bb": b_bcast,
        })
    return in_maps


def assemble_output(cfg: Cfg, outs):
    """Per-core 'out' arrays [P, TILES*(S+1)] -> full logits [1, N + 2*S*N].

    Slot order within a tile: 0..S-1 = samples, S = sc1.
    """
    S = cfg.s
    n = cfg.nodes_per_core * cfg.n_cores
    sc1 = np.empty((n,), np.float32)
    sc2 = np.empty((S, n), np.float32)
    for c in range(cfg.n_cores):
        o = (outs[c].reshape(P, cfg.tiles, S + 1).transpose(2, 1, 0)
             .reshape(S + 1, cfg.npad)[:, : cfg.nodes_per_core])
        lo = c * cfg.nodes_per_core
        sc1[lo:lo + cfg.nodes_per_core] = o[S]
        sc2[:, lo:lo + cfg.nodes_per_core] = o[:S]
    flat = sc2.reshape(-1)
    return np.concatenate([sc1, flat, flat])[None, :].astype(np.float32)


_NC_CACHE = {}


def _get_nc(cfg: Cfg):
    key = (cfg.n_table, cfg.nodes_per_core, cfg.d, cfg.s, cfg.super_tile)
    if key not in _NC_CACHE:
        _NC_CACHE[key] = build_nc(cfg)
    return _NC_CACHE[key]


def run_on_hw(cfg: Cfg, inputs, trace=False, trace_kwargs={}):
    nc = _get_nc(cfg)
    in_maps = make_in_maps(cfg, **inputs)
    res = run_bass_kernel_spmd(nc, in_maps, core_ids=list(range(cfg.n_cores)),
                               trace=trace, trace_kwargs=trace_kwargs)
    out = assemble_output(cfg, [r["out"] for r in res.results])
    return out, res


def kernel(h_c, h_pl, sample_list, W, b):
    inputs = dict(h_c=h_c, h_pl=h_pl, sample_list=sample_list, W=W, b=b)
    out, _ = run_on_hw(FULL, inputs, trace=False)
    return out


# revision 17
# speedup vs baseline: 1.1416x; 1.1108x over previous
"""Trainium2 Bass kernel for nn_Discriminator2 (bilinear discriminator scores).

Math: with hc0 = h_c[0] [N, D], W0 = W[0] [D, D]:
    v      = hc0 @ W0.T                      [N, D]   (tensor engine)
    sc1[n] = dot(h_pl[0][n], v[n]) + b       [N]      (fused DVE mult+reduce)
    sc2[s,n] = dot(hc0[sample[s,n]], v[n]) + b        (indirect-DMA gather + DVE)
    out    = [sc1 | sc2.flat | sc2.flat]     [1, N + 2*S*N]

Sharding: nodes (N) split evenly across 8 cores; hc0 replicated on every core
so gathers resolve locally; W replicated; h_pl / sample_list sharded by node.
"""

import sys

for _p in ("/opt/trn_rl_repo",):
    if _p not in sys.path:
        sys.path.insert(0, _p)

import numpy as np

import concourse.bass as bass
import concourse.mybir as mybir
import concourse.tile as tile
from concourse import bacc
from concourse.bass_utils import run_bass_kernel_spmd

P = 128  # partitions


class Cfg:
    """Problem geometry. Full-size defaults; shrink for CoreSim validation."""

    def __init__(self, n_table=100000, nodes_per_core=12500, d=512, s=4,
                 n_cores=8, super_tile=4, mm_dtype=mybir.dt.float32r):
        self.n_table = n_table          # rows of the gather table (full N)
        self.nodes_per_core = nodes_per_core
        self.d = d
        self.s = s
        self.n_cores = n_cores
        self.super_tile = super_tile    # node-tiles per hcT DMA block
        self.mm_dtype = mm_dtype
        self.tiles = -(-nodes_per_core // P)        # ceil
        self.npad = self.tiles * P
        self.kc = d // P                # contraction chunks


FULL = Cfg()


def build_nc(cfg: Cfg):
    D, S, KC, TILES = cfg.d, cfg.s, cfg.kc, cfg.tiles
    mmdt = cfg.mm_dtype
    f32 = mybir.dt.float32

    nc = bacc.Bacc("TRN2", target_bir_lowering=False, debug=False)
    hc = nc.dram_tensor("hc", [cfg.n_table, D], f32, kind="ExternalInput").ap()
    hcT = nc.dram_tensor("hcT", [D, cfg.npad], mmdt, kind="ExternalInput").ap()
    hpl = nc.dram_tensor("hpl", [cfg.npad, D], f32, kind="ExternalInput").ap()
    idx = nc.dram_tensor("idx", [P, TILES * S], mybir.dt.int32,
                         kind="ExternalInput").ap()
    wt = nc.dram_tensor("wt", [D, D], mmdt, kind="ExternalInput").ap()
    bb = nc.dram_tensor("bb", [P, 1], f32, kind="ExternalInput").ap()
    out = nc.dram_tensor("out", [P, TILES * (S + 1)], f32,
                         kind="ExternalOutput").ap()

    with tile.TileContext(nc) as tc:
        with (
            tc.tile_pool(name="const", bufs=1) as cpool,
            tc.tile_pool(name="hcT", bufs=2) as hcT_pool,
            tc.tile_pool(name="hpl", bufs=3) as hpl_pool,
            tc.tile_pool(name="g", bufs=3) as g_pool,
            tc.tile_pool(name="prod", bufs=6) as prod_pool,
            tc.tile_pool(name="psum", bufs=4, space="PSUM") as psum_pool,
        ):
            # W.T resident: free layout (c, d) — chunk c covers contraction
            # rows c*128..c*128+127.
            wt_sb = cpool.tile([P, KC * D], mmdt)
            nc.sync.dma_start(
                out=wt_sb[:].rearrange("p (c d) -> p c d", c=KC),
                in_=wt.rearrange("(c p) d -> p c d", p=P))
            # All gather indices resident: idx_sb[p, t*S+s] = sample[s, t*128+p].
            idx_sb = cpool.tile([P, TILES * S], mybir.dt.int32)
            nc.sync.dma_start(out=idx_sb[:], in_=idx[:])
            b_sb = cpool.tile([P, 1], f32)
            nc.sync.dma_start(out=b_sb[:], in_=bb[:])
            sc_acc = cpool.tile([P, TILES * (S + 1)], f32)
            dump = cpool.tile([P, D], f32)  # discarded ACT elementwise output

            for t0 in range(0, TILES, cfg.super_tile):
                st = min(cfg.super_tile, TILES - t0)
                # hcT block [D, st*128] -> SBUF free layout (c, n_local)
                hcT_sb = hcT_pool.tile([P, KC * cfg.super_tile * P], mmdt,
                                       tag="hcT")
                nc.sync.dma_start(
                    out=hcT_sb[:, : KC * st * P].rearrange(
                        "p (c n) -> p c n", c=KC),
                    in_=hcT[:, t0 * P:(t0 + st) * P].rearrange(
                        "(c p) n -> p c n", p=P),
                )
                for j in range(st):
                    t = t0 + j
                    hpl_sb = hpl_pool.tile([P, D], f32, tag="hpl")
                    nc.sync.dma_start(out=hpl_sb[:],
                                      in_=hpl[t * P:(t + 1) * P, :])
                    # Gather the S sampled rows per node (HW indirect DMA
                    # honors one index per partition, so one call per s):
                    # g_sb[p, s*D:(s+1)*D] = hc[idx_sb[p, t*S+s], :]
                    g_sb = g_pool.tile([P, S * D], f32, tag="g")
                    for s in range(S):
                        nc.gpsimd.indirect_dma_start(
                            out=g_sb[:, s * D:(s + 1) * D],
                            out_offset=None,
                            in_=hc[:],
                            in_offset=bass.IndirectOffsetOnAxis(
                                ap=idx_sb[:, t * S + s:t * S + s + 1], axis=0),
                        )
                    # v = hc0_tile @ W.T via 4 accumulating matmuls
                    v_ps = psum_pool.tile([P, D], f32, space="PSUM", tag="v_ps")
                    for c in range(KC):
                        off = (c * st + j) * P
                        nc.tensor.matmul(
                            out=v_ps[:],
                            lhsT=hcT_sb[:, off:off + P],
                            rhs=wt_sb[:, c * D:(c + 1) * D],
                            start=(c == 0),
                            stop=(c == KC - 1),
                        )
                    # 5 dot products: DVE multiplies (v read straight from
                    # PSUM), ScalarE reduces via Copy-activation accum_out.
                    for s in range(S + 1):
                        in0 = hpl_sb[:] if s == 0 else g_sb[:, (s - 1) * D:s * D]
                        prod = prod_pool.tile([P, D], f32, tag="prod")
                        nc.vector.tensor_mul(prod[:], in0, v_ps[:])
                        nc.scalar.activation(
                            dump[:], prod[:],
                            mybir.ActivationFunctionType.Copy,
                            accum_out=sc_acc[:, t * (S + 1) + s:
                                             t * (S + 1) + s + 1],
                        )
            nc.vector.tensor_scalar_add(sc_acc[:], sc_acc[:], b_sb[:, :1])
            nc.sync.dma_start(out=out[:], in_=sc_acc[:])
    nc.compile()
    return nc


def make_in_maps(cfg: Cfg, h_c, h_pl, sample_list, W, b):
    """Host-side sharding: full inputs -> per-core input dicts."""
    D, S = cfg.d, cfg.s
    hc0 = np.ascontiguousarray(np.asarray(h_c, np.float32)[0])
    hpl0 = np.asarray(h_pl, np.float32)[0]
    smp = np.asarray(sample_list)
    W0 = np.asarray(W, np.float32)[0]
    bval = float(np.asarray(b, np.float32).reshape(-1)[0])

    hcT = np.ascontiguousarray(hc0.T)                  # [D, N]
    wt = np.ascontiguousarray(W0.T)                    # wt[e, d] = W[d, e]
    b_bcast = np.full((P, 1), bval, np.float32)

    in_maps = []
    for c in range(cfg.n_cores):
        lo = c * cfg.nodes_per_core
        hi = lo + cfg.nodes_per_core
        hcT_s = np.zeros((D, cfg.npad), np.float32)
        hcT_s[:, : cfg.nodes_per_core] = hcT[:, lo:hi]
        hpl_s = np.zeros((cfg.npad, D), np.float32)
        hpl_s[: cfg.nodes_per_core] = hpl0[lo:hi]
        idx_s = np.zeros((S, cfg.npad), np.int64)
        idx_s[:, : cfg.nodes_per_core] = smp[:, lo:hi]
        idx_r = np.ascontiguousarray(
            idx_s.reshape(S, cfg.tiles, P).transpose(2, 1, 0)
            .astype(np.int32).reshape(P, cfg.tiles * S))
        in_maps.append({
            "hc": hc0, "hcT": hcT_s, "hpl": hpl_s,
            "idx": idx_r, "wt": wt, "bb": b_bcast,
        })
    return in_maps


def assemble_output(cfg: Cfg, outs):
    """Per-core 'out' arrays [P, TILES*(S+1)] -> full logits [1, N + 2*S*N]."""
    S = cfg.s
    n = cfg.nodes_per_core * cfg.n_cores
    sc1 = np.empty((n,), np.float32)
    sc2 = np.empty((S, n), np.float32)
    for c in range(cfg.n_cores):
        o = (outs[c].reshape(P, cfg.tiles, S + 1).transpose(2, 1, 0)
             .reshape(S + 1, cfg.npad)[:, : cfg.nodes_per_core])
        lo = c * cfg.nodes_per_core
        sc1[lo:lo + cfg.nodes_per_core] = o[0]
        sc2[:, lo:lo + cfg.nodes_per_core] = o[1:]
    flat = sc2.reshape(-1)
    return np.concatenate([sc1, flat, flat])[None, :].astype(np.float32)


_NC_CACHE = {}


def _get_nc(cfg: Cfg):
    key = (cfg.n_table, cfg.nodes_per_core, cfg.d, cfg.s, cfg.super_tile,
           cfg.mm_dtype)
    if key not in _NC_CACHE:
        _NC_CACHE[key] = build_nc(cfg)
    return _NC_CACHE[key]


def run_on_hw(cfg: Cfg, inputs, trace=False, trace_kwargs={}):
    nc = _get_nc(cfg)
    in_maps = make_in_maps(cfg, **inputs)
    res = run_bass_kernel_spmd(nc, in_maps, core_ids=list(range(cfg.n_cores)),
                               trace=trace, trace_kwargs=trace_kwargs)
    out = assemble_output(cfg, [r["out"] for r in res.results])
    return out, res


def kernel(h_c, h_pl, sample_list, W, b):
    inputs = dict(h_c=h_c, h_pl=h_pl, sample_list=sample_list, W=W, b=b)
    out, _ = run_on_hw(FULL, inputs, trace=False)
    return out


# revision 19
# speedup vs baseline: 1.1468x; 1.0046x over previous
"""Trainium2 Bass kernel for nn_Discriminator2 (bilinear discriminator scores).

Math: with hc0 = h_c[0] [N, D], W0 = W[0] [D, D]:
    v      = hc0 @ W0.T                      [N, D]   (tensor engine)
    sc1[n] = dot(h_pl[0][n], v[n]) + b       [N]      (fused DVE mult+reduce)
    sc2[s,n] = dot(hc0[sample[s,n]], v[n]) + b        (indirect-DMA gather + DVE)
    out    = [sc1 | sc2.flat | sc2.flat]     [1, N + 2*S*N]

Sharding: nodes (N) split evenly across 8 cores; hc0 replicated on every core
so gathers resolve locally; W replicated; h_pl / sample_list sharded by node.
"""

import sys

for _p in ("/opt/trn_rl_repo",):
    if _p not in sys.path:
        sys.path.insert(0, _p)

import numpy as np

import concourse.bass as bass
import concourse.mybir as mybir
import concourse.tile as tile
from concourse import bacc
from concourse.bass_utils import run_bass_kernel_spmd

P = 128  # partitions


class Cfg:
    """Problem geometry. Full-size defaults; shrink for CoreSim validation."""

    def __init__(self, n_table=100000, nodes_per_core=12500, d=512, s=4,
                 n_cores=8, super_tile=4, mm_dtype=mybir.dt.float32r):
        self.n_table = n_table          # rows of the gather table (full N)
        self.nodes_per_core = nodes_per_core
        self.d = d
        self.s = s
        self.n_cores = n_cores
        self.super_tile = super_tile    # node-tiles per hcT DMA block
        self.mm_dtype = mm_dtype
        self.tiles = -(-nodes_per_core // P)        # ceil
        self.npad = self.tiles * P
        self.kc = d // P                # contraction chunks


FULL = Cfg()


def build_nc(cfg: Cfg):
    D, S, KC, TILES = cfg.d, cfg.s, cfg.kc, cfg.tiles
    mmdt = cfg.mm_dtype
    f32 = mybir.dt.float32

    nc = bacc.Bacc("TRN2", target_bir_lowering=False, debug=False,
                   num_swdge_queues=2)
    hc = nc.dram_tensor("hc", [cfg.n_table, D], f32, kind="ExternalInput").ap()
    hcT = nc.dram_tensor("hcT", [D, cfg.npad], mmdt, kind="ExternalInput").ap()
    hpl = nc.dram_tensor("hpl", [cfg.npad, D], f32, kind="ExternalInput").ap()
    idx = nc.dram_tensor("idx", [P, TILES * S], mybir.dt.int32,
                         kind="ExternalInput").ap()
    wt = nc.dram_tensor("wt", [D, D], mmdt, kind="ExternalInput").ap()
    bb = nc.dram_tensor("bb", [P, 1], f32, kind="ExternalInput").ap()
    out = nc.dram_tensor("out", [P, TILES * (S + 1)], f32,
                         kind="ExternalOutput").ap()

    with tile.TileContext(nc) as tc:
        with (
            tc.tile_pool(name="const", bufs=1) as cpool,
            tc.tile_pool(name="hcT", bufs=2) as hcT_pool,
            tc.tile_pool(name="hpl", bufs=3) as hpl_pool,
            tc.tile_pool(name="g", bufs=3) as g_pool,
            tc.tile_pool(name="prod", bufs=6) as prod_pool,
            tc.tile_pool(name="psum", bufs=4, space="PSUM") as psum_pool,
        ):
            # W.T resident: free layout (c, d) — chunk c covers contraction
            # rows c*128..c*128+127.
            wt_sb = cpool.tile([P, KC * D], mmdt)
            nc.sync.dma_start(
                out=wt_sb[:].rearrange("p (c d) -> p c d", c=KC),
                in_=wt.rearrange("(c p) d -> p c d", p=P))
            # All gather indices resident: idx_sb[p, t*S+s] = sample[s, t*128+p].
            idx_sb = cpool.tile([P, TILES * S], mybir.dt.int32)
            nc.sync.dma_start(out=idx_sb[:], in_=idx[:])
            b_sb = cpool.tile([P, 1], f32)
            nc.sync.dma_start(out=b_sb[:], in_=bb[:])
            sc_acc = cpool.tile([P, TILES * (S + 1)], f32)
            dump = cpool.tile([P, D], f32)  # discarded ACT elementwise output

            for t0 in range(0, TILES, cfg.super_tile):
                st = min(cfg.super_tile, TILES - t0)
                # hcT block [D, st*128] -> SBUF free layout (c, n_local)
                hcT_sb = hcT_pool.tile([P, KC * cfg.super_tile * P], mmdt,
                                       tag="hcT")
                nc.sync.dma_start(
                    out=hcT_sb[:, : KC * st * P].rearrange(
                        "p (c n) -> p c n", c=KC),
                    in_=hcT[:, t0 * P:(t0 + st) * P].rearrange(
                        "(c p) n -> p c n", p=P),
                )
                for j in range(st):
                    t = t0 + j
                    hpl_sb = hpl_pool.tile([P, D], f32, tag="hpl")
                    nc.sync.dma_start(out=hpl_sb[:],
                                      in_=hpl[t * P:(t + 1) * P, :])
                    # Gather the S sampled rows per node (HW indirect DMA
                    # honors one index per partition, so one call per s):
                    # g_sb[p, s*D:(s+1)*D] = hc[idx_sb[p, t*S+s], :]
                    g_sb = g_pool.tile([P, S * D], f32, tag="g")
                    for s in range(S):
                        gi = nc.gpsimd.indirect_dma_start(
                            out=g_sb[:, s * D:(s + 1) * D],
                            out_offset=None,
                            in_=hc[:],
                            in_offset=bass.IndirectOffsetOnAxis(
                                ap=idx_sb[:, t * S + s:t * S + s + 1], axis=0),
                        )
                        # alternate SWDGE queues so SDMA interleaves two
                        # descriptor streams (hides random-row HBM latency)
                        if s % 2 == 1:
                            gi.ins.queue = "qPoolDynamic1"
                    # v = hc0_tile @ W.T via 4 accumulating matmuls
                    v_ps = psum_pool.tile([P, D], f32, space="PSUM", tag="v_ps")
                    for c in range(KC):
                        off = (c * st + j) * P
                        nc.tensor.matmul(
                            out=v_ps[:],
                            lhsT=hcT_sb[:, off:off + P],
                            rhs=wt_sb[:, c * D:(c + 1) * D],
                            start=(c == 0),
                            stop=(c == KC - 1),
                        )
                    # 5 dot products: DVE multiplies (v read straight from
                    # PSUM), ScalarE reduces via Copy-activation accum_out.
                    for s in range(S + 1):
                        in0 = hpl_sb[:] if s == 0 else g_sb[:, (s - 1) * D:s * D]
                        prod = prod_pool.tile([P, D], f32, tag="prod")
                        nc.vector.tensor_mul(prod[:], in0, v_ps[:])
                        nc.scalar.activation(
                            dump[:], prod[:],
                            mybir.ActivationFunctionType.Copy,
                            accum_out=sc_acc[:, t * (S + 1) + s:
                                             t * (S + 1) + s + 1],
                        )
            nc.vector.tensor_scalar_add(sc_acc[:], sc_acc[:], b_sb[:, :1])
            nc.sync.dma_start(out=out[:], in_=sc_acc[:])
    nc.compile()
    return nc


def make_in_maps(cfg: Cfg, h_c, h_pl, sample_list, W, b):
    """Host-side sharding: full inputs -> per-core input dicts."""
    D, S = cfg.d, cfg.s
    hc0 = np.ascontiguousarray(np.asarray(h_c, np.float32)[0])
    hpl0 = np.asarray(h_pl, np.float32)[0]
    smp = np.asarray(sample_list)
    W0 = np.asarray(W, np.float32)[0]
    bval = float(np.asarray(b, np.float32).reshape(-1)[0])

    hcT = np.ascontiguousarray(hc0.T)                  # [D, N]
    wt = np.ascontiguousarray(W0.T)                    # wt[e, d] = W[d, e]
    b_bcast = np.full((P, 1), bval, np.float32)

    in_maps = []
    for c in range(cfg.n_cores):
        lo = c * cfg.nodes_per_core
        hi = lo + cfg.nodes_per_core
        hcT_s = np.zeros((D, cfg.npad), np.float32)
        hcT_s[:, : cfg.nodes_per_core] = hcT[:, lo:hi]
        hpl_s = np.zeros((cfg.npad, D), np.float32)
        hpl_s[: cfg.nodes_per_core] = hpl0[lo:hi]
        idx_s = np.zeros((S, cfg.npad), np.int64)
        idx_s[:, : cfg.nodes_per_core] = smp[:, lo:hi]
        idx_r = np.ascontiguousarray(
            idx_s.reshape(S, cfg.tiles, P).transpose(2, 1, 0)
            .astype(np.int32).reshape(P, cfg.tiles * S))
        in_maps.append({
            "hc": hc0, "hcT": hcT_s, "hpl": hpl_s,
            "idx": idx_r, "wt": wt, "bb": b_bcast,
        })
    return in_maps


def assemble_output(cfg: Cfg, outs):
    """Per-core 'out' arrays [P, TILES*(S+1)] -> full logits [1, N + 2*S*N]."""
    S = cfg.s
    n = cfg.nodes_per_core * cfg.n_cores
    sc1 = np.empty((n,), np.float32)
    sc2 = np.empty((S, n), np.float32)
    for c in range(cfg.n_cores):
        o = (outs[c].reshape(P, cfg.tiles, S + 1).transpose(2, 1, 0)
             .reshape(S + 1, cfg.npad)[:, : cfg.nodes_per_core])
        lo = c * cfg.nodes_per_core
        sc1[lo:lo + cfg.nodes_per_core] = o[0]
        sc2[:, lo:lo + cfg.nodes_per_core] = o[1:]
    flat = sc2.reshape(-1)
    return np.concatenate([sc1, flat, flat])[None, :].astype(np.float32)


_NC_CACHE = {}


def _get_nc(cfg: Cfg):
    key = (cfg.n_table, cfg.nodes_per_core, cfg.d, cfg.s, cfg.super_tile,
           cfg.mm_dtype)
    if key not in _NC_CACHE:
        _NC_CACHE[key] = build_nc(cfg)
    return _NC_CACHE[key]


def run_on_hw(cfg: Cfg, inputs, trace=False, trace_kwargs={}):
    nc = _get_nc(cfg)
    in_maps = make_in_maps(cfg, **inputs)
    res = run_bass_kernel_spmd(nc, in_maps, core_ids=list(range(cfg.n_cores)),
                               trace=trace, trace_kwargs=trace_kwargs)
    out = assemble_output(cfg, [r["out"] for r in res.results])
    return out, res


def kernel(h_c, h_pl, sample_list, W, b):
    inputs = dict(h_c=h_c, h_pl=h_pl, sample_list=sample_list, W=W, b=b)
    out, _ = run_on_hw(FULL, inputs, trace=False)
    return out


# revision 20
# speedup vs baseline: 1.1509x; 1.0036x over previous
"""Trainium2 Bass kernel for nn_Discriminator2 (bilinear discriminator scores).

Math: with hc0 = h_c[0] [N, D], W0 = W[0] [D, D]:
    v      = hc0 @ W0.T                      [N, D]   (tensor engine)
    sc1[n] = dot(h_pl[0][n], v[n]) + b       [N]      (fused DVE mult+reduce)
    sc2[s,n] = dot(hc0[sample[s,n]], v[n]) + b        (indirect-DMA gather + DVE)
    out    = [sc1 | sc2.flat | sc2.flat]     [1, N + 2*S*N]

Sharding: nodes (N) split evenly across 8 cores; hc0 replicated on every core
so gathers resolve locally; W replicated; h_pl / sample_list sharded by node.
"""

import sys

for _p in ("/opt/trn_rl_repo",):
    if _p not in sys.path:
        sys.path.insert(0, _p)

import numpy as np

import concourse.bass as bass
import concourse.mybir as mybir
import concourse.tile as tile
from concourse import bacc
from concourse.bass_utils import run_bass_kernel_spmd

P = 128  # partitions


class Cfg:
    """Problem geometry. Full-size defaults; shrink for CoreSim validation."""

    def __init__(self, n_table=100000, nodes_per_core=12500, d=512, s=4,
                 n_cores=8, super_tile=4, mm_dtype=mybir.dt.float32r):
        self.n_table = n_table          # rows of the gather table (full N)
        self.nodes_per_core = nodes_per_core
        self.d = d
        self.s = s
        self.n_cores = n_cores
        self.super_tile = super_tile    # node-tiles per hcT DMA block
        self.mm_dtype = mm_dtype
        self.tiles = -(-nodes_per_core // P)        # ceil
        self.npad = self.tiles * P
        self.kc = d // P                # contraction chunks


FULL = Cfg()


def build_nc(cfg: Cfg):
    D, S, KC, TILES = cfg.d, cfg.s, cfg.kc, cfg.tiles
    mmdt = cfg.mm_dtype
    f32 = mybir.dt.float32

    nc = bacc.Bacc("TRN2", target_bir_lowering=False, debug=False,
                   num_swdge_queues=2)
    hc = nc.dram_tensor("hc", [cfg.n_table, D], f32, kind="ExternalInput").ap()
    hcT = nc.dram_tensor("hcT", [D, cfg.npad], mmdt, kind="ExternalInput").ap()
    hpl = nc.dram_tensor("hpl", [cfg.npad, D], f32, kind="ExternalInput").ap()
    idx = nc.dram_tensor("idx", [P, TILES * S], mybir.dt.int32,
                         kind="ExternalInput").ap()
    wt = nc.dram_tensor("wt", [D, D], mmdt, kind="ExternalInput").ap()
    bb = nc.dram_tensor("bb", [P, 1], f32, kind="ExternalInput").ap()
    out = nc.dram_tensor("out", [P, TILES * (S + 1)], f32,
                         kind="ExternalOutput").ap()

    with tile.TileContext(nc) as tc:
        with (
            tc.tile_pool(name="const", bufs=1) as cpool,
            tc.tile_pool(name="hcT", bufs=2) as hcT_pool,
            tc.tile_pool(name="hpl", bufs=4) as hpl_pool,
            tc.tile_pool(name="g", bufs=6) as g_pool,
            tc.tile_pool(name="prod", bufs=8) as prod_pool,
            tc.tile_pool(name="psum", bufs=4, space="PSUM") as psum_pool,
        ):
            # All gather indices resident: idx_sb[p, t*S+s] = sample[s, t*128+p].
            # Loaded FIRST so the gather stream (the kernel's critical path)
            # starts as early as possible.
            idx_sb = cpool.tile([P, TILES * S], mybir.dt.int32)
            nc.sync.dma_start(out=idx_sb[:], in_=idx[:])
            # W.T resident: free layout (c, d) — chunk c covers contraction
            # rows c*128..c*128+127.
            wt_sb = cpool.tile([P, KC * D], mmdt)
            nc.sync.dma_start(
                out=wt_sb[:].rearrange("p (c d) -> p c d", c=KC),
                in_=wt.rearrange("(c p) d -> p c d", p=P))
            b_sb = cpool.tile([P, 1], f32)
            nc.sync.dma_start(out=b_sb[:], in_=bb[:])
            sc_acc = cpool.tile([P, TILES * (S + 1)], f32)
            dump = cpool.tile([P, D], f32)  # discarded ACT elementwise output

            for t0 in range(0, TILES, cfg.super_tile):
                st = min(cfg.super_tile, TILES - t0)
                # hcT block [D, st*128] -> SBUF free layout (c, n_local)
                hcT_sb = hcT_pool.tile([P, KC * cfg.super_tile * P], mmdt,
                                       tag="hcT")
                nc.sync.dma_start(
                    out=hcT_sb[:, : KC * st * P].rearrange(
                        "p (c n) -> p c n", c=KC),
                    in_=hcT[:, t0 * P:(t0 + st) * P].rearrange(
                        "(c p) n -> p c n", p=P),
                )
                for j in range(st):
                    t = t0 + j
                    hpl_sb = hpl_pool.tile([P, D], f32, tag="hpl")
                    nc.sync.dma_start(out=hpl_sb[:],
                                      in_=hpl[t * P:(t + 1) * P, :])
                    # Gather the S sampled rows per node (HW indirect DMA
                    # honors one index per partition, so one call per s):
                    # g_sb[p, s*D:(s+1)*D] = hc[idx_sb[p, t*S+s], :]
                    g_sb = g_pool.tile([P, S * D], f32, tag="g")
                    for s in range(S):
                        gi = nc.gpsimd.indirect_dma_start(
                            out=g_sb[:, s * D:(s + 1) * D],
                            out_offset=None,
                            in_=hc[:],
                            in_offset=bass.IndirectOffsetOnAxis(
                                ap=idx_sb[:, t * S + s:t * S + s + 1], axis=0),
                        )
                        # alternate SWDGE queues so SDMA interleaves two
                        # descriptor streams (hides random-row HBM latency)
                        if s % 2 == 1:
                            gi.ins.queue = "qPoolDynamic1"
                    # v = hc0_tile @ W.T via 4 accumulating matmuls
                    v_ps = psum_pool.tile([P, D], f32, space="PSUM", tag="v_ps")
                    for c in range(KC):
                        off = (c * st + j) * P
                        nc.tensor.matmul(
                            out=v_ps[:],
                            lhsT=hcT_sb[:, off:off + P],
                            rhs=wt_sb[:, c * D:(c + 1) * D],
                            start=(c == 0),
                            stop=(c == KC - 1),
                        )
                    # 5 dot products: DVE multiplies (v read straight from
                    # PSUM), ScalarE reduces via Copy-activation accum_out.
                    for s in range(S + 1):
                        in0 = hpl_sb[:] if s == 0 else g_sb[:, (s - 1) * D:s * D]
                        prod = prod_pool.tile([P, D], f32, tag="prod")
                        nc.vector.tensor_mul(prod[:], in0, v_ps[:])
                        nc.scalar.activation(
                            dump[:], prod[:],
                            mybir.ActivationFunctionType.Copy,
                            accum_out=sc_acc[:, t * (S + 1) + s:
                                             t * (S + 1) + s + 1],
                        )
            nc.vector.tensor_scalar_add(sc_acc[:], sc_acc[:], b_sb[:, :1])
            nc.sync.dma_start(out=out[:], in_=sc_acc[:])
    nc.compile()
    return nc


def make_in_maps(cfg: Cfg, h_c, h_pl, sample_list, W, b):
    """Host-side sharding: full inputs -> per-core input dicts."""
    D, S = cfg.d, cfg.s
    hc0 = np.ascontiguousarray(np.asarray(h_c, np.float32)[0])
    hpl0 = np.asarray(h_pl, np.float32)[0]
    smp = np.asarray(sample_list)
    W0 = np.asarray(W, np.float32)[0]
    bval = float(np.asarray(b, np.float32).reshape(-1)[0])

    hcT = np.ascontiguousarray(hc0.T)                  # [D, N]
    wt = np.ascontiguousarray(W0.T)                    # wt[e, d] = W[d, e]
    b_bcast = np.full((P, 1), bval, np.float32)

    in_maps = []
    for c in range(cfg.n_cores):
        lo = c * cfg.nodes_per_core
        hi = lo + cfg.nodes_per_core
        hcT_s = np.zeros((D, cfg.npad), np.float32)
        hcT_s[:, : cfg.nodes_per_core] = hcT[:, lo:hi]
        hpl_s = np.zeros((cfg.npad, D), np.float32)
        hpl_s[: cfg.nodes_per_core] = hpl0[lo:hi]
        idx_s = np.zeros((S, cfg.npad), np.int64)
        idx_s[:, : cfg.nodes_per_core] = smp[:, lo:hi]
        idx_r = np.ascontiguousarray(
            idx_s.reshape(S, cfg.tiles, P).transpose(2, 1, 0)
            .astype(np.int32).reshape(P, cfg.tiles * S))
        in_maps.append({
            "hc": hc0, "hcT": hcT_s, "hpl": hpl_s,
            "idx": idx_r, "wt": wt, "bb": b_bcast,
        })
    return in_maps


def assemble_output(cfg: Cfg, outs):
    """Per-core 'out' arrays [P, TILES*(S+1)] -> full logits [1, N + 2*S*N]."""
    S = cfg.s
    n = cfg.nodes_per_core * cfg.n_cores
    sc1 = np.empty((n,), np.float32)
    sc2 = np.empty((S, n), np.float32)
    for c in range(cfg.n_cores):
        o = (outs[c].reshape(P, cfg.tiles, S + 1).transpose(2, 1, 0)
             .reshape(S + 1, cfg.npad)[:, : cfg.nodes_per_core])
        lo = c * cfg.nodes_per_core
        sc1[lo:lo + cfg.nodes_per_core] = o[0]
        sc2[:, lo:lo + cfg.nodes_per_core] = o[1:]
    flat = sc2.reshape(-1)
    return np.concatenate([sc1, flat, flat])[None, :].astype(np.float32)


_NC_CACHE = {}


def _get_nc(cfg: Cfg):
    key = (cfg.n_table, cfg.nodes_per_core, cfg.d, cfg.s, cfg.super_tile,
           cfg.mm_dtype)
    if key not in _NC_CACHE:
        _NC_CACHE[key] = build_nc(cfg)
    return _NC_CACHE[key]


def run_on_hw(cfg: Cfg, inputs, trace=False, trace_kwargs={}):
    nc = _get_nc(cfg)
    in_maps = make_in_maps(cfg, **inputs)
    res = run_bass_kernel_spmd(nc, in_maps, core_ids=list(range(cfg.n_cores)),
                               trace=trace, trace_kwargs=trace_kwargs)
    out = assemble_output(cfg, [r["out"] for r in res.results])
    return out, res


def kernel(h_c, h_pl, sample_list, W, b):
    inputs = dict(h_c=h_c, h_pl=h_pl, sample_list=sample_list, W=W, b=b)
    out, _ = run_on_hw(FULL, inputs, trace=False)
    return out
